# revision 16
# baseline (speedup 1.0000x reference)
"""CompressKV gating kernel for 8 Trainium2 NeuronCores.

Reference computation (per batch b, head h):
    x_s = x[b, :, h, :]                                  # [N=4096, D=128]
    windows n = 0..254, rows r = 16n + k, k = 0..31
    logits[n, g] = sum_{k,d} x_s[16n+k, d] * W[g, k, d]  # W = W_gate.reshape(32,32,128)
    gate = softmax_g(logits)
    out[n, d] = sum_k gate[n, k] * x_s[16n+k, d]

Sharding: B*H = 32 (b,h) slices, 4 per core, data/tensor parallel, no
cross-core communication.  Host pre-packs x per core in two bf16 layouts:
  xn: window-chunked native  [4, 128(p), 32(c)*128(d)]  (chunk c = rows 128c+p)
  xt: d-major (transposed)   [4, 128(d), 4224(n pad)]
plus the gate weight transposed to d-major wt[d, k*32+g].

HBM input traffic (8.5 MB/core, ~330 GB/s on the SP ring) is the hard
floor.  All input DMAs are issued up-front in an order matched to the
compute schedule (xt0 xt1 xn0 xt2 xn1 xt3 xn2 xn3) so the PE never waits
more than ~1.5 us and the HAM clock gate stays warm.

Softmax normalization is NOT done on device: the kernel ships the
unnormalized pooled output outT[d, n] = sum_k e[k,n] x[16n+k, d] (bf16)
plus the denominators den[n] = sum_k e[k,n] (f32), and the host divides.
This removes the recip/broadcast/mul chain from the critical path.

On-device pipeline per slice:
  A) logits via 32 accumulating matmuls (contract d chunks per k), 4 k's
     packed concurrently into the PE array via col-tiling -> psA[(kg,g), n]
  B) fold 4 col-groups (ACT copy + 3 DVE adds), exp (ACT) -> e[32, n] bf16,
     den via ones-matmul -> ACT copy to a packed SBUF strip.
  C) banded-matrix pooling: S[r, window] built from e with 9 band matmuls;
     32 matmuls with x chunks stationary -> psC[d, n]; DVE copy -> bf16;
     per-slice DMA on the DVE ring right after the copy.
PE instruction order interleaves slices (A_s | den/S_s | C_{s-1}) so
stage C of one slice fills the fold/exp latency of the next.
"""

import sys

import numpy as np

for _p in ("/opt/trn_rl_repo", "/opt/pypackages"):
    if _p not in sys.path:
        sys.path.append(_p)

import ml_dtypes

_B, _N, _H, _D = 2, 4096, 16, 128
_K = 32          # window (kernel) size
_ST = 16         # stride
_NB = 255        # num windows
_NC = 8          # cores
_SL = 4          # (b,h) slices per core
_NT = 4224       # padded n extent for xt (>= 16*255+31+1)
_NCH = 32        # 128-row chunks per slice

_prog_cache = {}


def _build_program():
    import concourse.mybir as mybir
    from concourse import bacc, tile

    f32 = mybir.dt.float32
    bf16 = mybir.dt.bfloat16
    AF = mybir.ActivationFunctionType

    nc = bacc.Bacc()
    xn = nc.dram_tensor("xn", [_SL, 128, _NCH * _D], bf16, kind="ExternalInput")
    xt = nc.dram_tensor("xt", [_SL, 128, _NT], bf16, kind="ExternalInput")
    wt = nc.dram_tensor("wt", [128, _K * _K], bf16, kind="ExternalInput")
    out = nc.dram_tensor("out", [_SL, _D, _NB], bf16, kind="ExternalOutput")
    outd = nc.dram_tensor("outd", [1, _SL * 256], f32, kind="ExternalOutput")

    with tile.TileContext(nc) as tc:
        with (
            tc.tile_pool(name="const", bufs=1) as cpool,
            tc.tile_pool(name="data", bufs=1) as dpool,
            tc.tile_pool(name="small", bufs=2) as spool,
            tc.tile_pool(name="psA", bufs=2, space="PSUM") as psa_pool,
            tc.tile_pool(name="psC", bufs=2, space="PSUM") as psc_pool,
            tc.tile_pool(name="psS", bufs=2, space="PSUM") as pss_pool,
            tc.tile_pool(name="psD", bufs=1, space="PSUM") as psd_pool,
            tc.tile_pool(name="psW", bufs=1, space="PSUM") as psw_pool,
        ):
            # ---- all input DMAs first, ordered to match the compute
            # schedule, so the SP ring streams back-to-back
            wt_sb = cpool.tile([128, _K * _K], bf16)
            nc.sync.dma_start(wt_sb[:], wt[:, :])
            # HAM warm-up: dependency-free matmuls on a memset constant tile
            # (ready ~4 us before the first DMA lands) keep the PE busy
            # through the preamble so the clock gate reaches 8/8 before the
            # first real matmul.
            wrm = cpool.tile([128, 128], bf16)
            nc.vector.memset(wrm[:], 0.5)
            psW = psw_pool.tile([32, 128], f32)
            for _w in range(45):
                nc.tensor.matmul(
                    psW[:, :], wrm[:, 0:32], wrm[:, :],
                    start=True, stop=True, skip_group_check=True,
                )
            xt_t = {
                s: [
                    dpool.tile([128, 2112], bf16, tag=f"xt{s}h{h}", name=f"xt{s}h{h}")
                    for h in range(2)
                ]
                for s in range(_SL)
            }
            xn_t = {
                s: [
                    dpool.tile([128, 16 * _D], bf16, tag=f"xn{s}h{h}", name=f"xn{s}h{h}")
                    for h in range(2)
                ]
                for s in range(_SL)
            }

            def load_xt(s):
                # xt half h covers window cols [2048h, 2048h+2112)
                nc.sync.dma_start(xt_t[s][0][:], xt[s, :, 0:2112])
                nc.sync.dma_start(xt_t[s][1][:], xt[s, :, 2048:4160])

            def load_xn(s):
                # halves keep the DMA partition lines at 4 KB (full rate);
                # stage C's chunk matmuls gate per-half via region deps.
                # xn rides the ACT HWDGE ring so the two physical rings
                # stream concurrently (single ring measured ~331 GB/s of
                # the ~358 GB/s HBM-per-core limit)
                nc.scalar.dma_start(xn_t[s][0][:], xn[s, :, 0 : 16 * _D])
                nc.scalar.dma_start(xn_t[s][1][:], xn[s, :, 16 * _D : 32 * _D])

            # all xt first: every logits/softmax chain completes with slack
            # while the xn stream is still arriving; the kernel tail is just
            # last-xn-quarter -> 8 pooling matmuls -> copy -> DMA
            for s in range(_SL):
                load_xt(s)
            for s in range(_SL):
                load_xn(s)

            # ---- constants
            ones32 = cpool.tile([32, 1], bf16)
            nc.vector.memset(ones32[:], 1.0)
            # band master: mband[k, c] = 1 iff c == k + 128.  Slicing cols
            # [144-16j : 272-16j] gives the [32k, 128r] band matrix with
            # 1 at r == k + 16j - 16 (rows outside [0,128) auto-dropped).
            mband = cpool.tile([32, 272], bf16)
            nc.gpsimd.memset(mband[:], 0.0)
            nc.gpsimd.affine_select(
                out=mband[:],
                in_=mband[:],
                compare_op=mybir.AluOpType.not_equal,
                fill=1.0,
                base=128,
                # fill where 128 + x - y == 0, i.e. y == x + 128
                pattern=[[-1, 272]],
                channel_multiplier=1,
            )
            den_all = cpool.tile([1, _SL * 256], f32)

            # ---- per-slice emitters (called in a hand-interleaved order) --
            psA_t, e_t, S_t, psC_t, outb_t = {}, {}, {}, {}, {}

            def emit_A(s):
                # logits via 32 accumulating matmuls per half (contract d
                # chunks per k), 4 k's packed concurrently into the PE
                # array via col-tiling (4-way streams measured ~2.5x the
                # serial rate, beating formulations with less traffic).
                psA = psa_pool.tile([128, 256], f32, tag="psA", name=f"psA_{s}")
                for h in range(2):
                    for t in range(8):
                        for j in range(4):
                            k = 4 * t + j
                            nc.tensor.matmul(
                                psA[32 * j : 32 * j + 32, 128 * h : 128 * h + 128],
                                wt_sb[:, 32 * k : 32 * k + 32],
                                xt_t[s][h][:, k : k + 16 * 128 : 16],
                                start=(t == 0),
                                stop=(t == 7),
                                tile_position=(0, 32 * j),
                                skip_group_check=True,
                            )
                psA_t[s] = psA

            def emit_fold(s):
                # fold the 4 col-groups: logits[g, n] = sum_j psA[32j+g, n]
                # (ACT evicts the first group; DVE may read at most one
                # PSUM operand per op.)  Then exp on ACT.
                # fold the 4 col-groups: logits[g, n] = sum_j psA[32j+g, n]
                # (ACT evicts the first group; DVE may read at most one PSUM
                # operand per op), then exp on ACT
                psA = psA_t[s]
                t0 = spool.tile([32, 256], f32, tag="t0", name=f"t0_{s}")
                t1 = spool.tile([32, 256], f32, tag="t1", name=f"t1_{s}")
                t2 = spool.tile([32, 256], f32, tag="t2", name=f"t2_{s}")
                logits = spool.tile([32, 256], f32, tag="lg", name=f"lg_{s}")
                nc.scalar.activation(t0[:], psA[0:32, :], AF.Copy)
                nc.vector.tensor_add(t1[:], t0[:], psA[32:64, :])
                nc.vector.tensor_add(t2[:], t1[:], psA[64:96, :])
                nc.vector.tensor_add(logits[:], t2[:], psA[96:128, :])
                e_kn = spool.tile([32, 256], bf16, tag="e", name=f"e_{s}")
                nc.scalar.activation(e_kn[:], logits[:], AF.Exp)
                e_t[s] = e_kn

            def emit_den_mm(s):
                psD = psd_pool.tile([32, 256], f32, tag="psD", name=f"psD_{s}")
                nc.tensor.matmul(
                    psD[0:1, :], ones32[:, 0:1], e_t[s][:, :],
                    start=True, stop=True,
                )
                return psD

            def emit_S_mm(s):
                # S matrix (class-major cols 32j + c): window n = 8c-1+j,
                # S[r=16j-16+k, 32j+c] = e[k, n].  Built on PE via band-
                # matrix lhsT (partition placement encoded in the matrix),
                # since engine partition bases must be 32-aligned.
                psS = pss_pool.tile([128, 9 * 32], f32, tag="psS", name=f"psS_{s}")
                for j in range(9):
                    c0 = 1 if j == 0 else 0
                    c1 = 31 if j == 8 else 32
                    nc.tensor.matmul(
                        psS[:, 32 * j + c0 : 32 * j + c1],
                        mband[:, 144 - 16 * j : 272 - 16 * j],
                        e_t[s][:, 8 * c0 + j - 1 : 8 * (c1 - 1) + j : 8],
                        start=True,
                        stop=True,
                        skip_group_check=True,
                    )
                return psS

            def emit_den_copy(s, psD):
                nc.scalar.activation(
                    den_all[0:1, 256 * s : 256 * s + 256], psD[0:1, :], AF.Copy
                )

            def emit_S_copy(s, psS):
                S_sb = spool.tile([128, 9 * 32], bf16, tag="S", name=f"S_{s}")
                # cols 0 and 287 are never written (invalid windows) nor read
                nc.vector.tensor_copy(S_sb[:, 1:287], psS[:, 1:287])
                S_t[s] = S_sb

            def emit_psC_memset(s):
                psC = psc_pool.tile([128, _NB], f32, tag="psC", name=f"psC_{s}")
                nc.vector.memset(psC[:], 0.0)
                psC_t[s] = psC

            def emit_C(s):
                psC = psC_t[s]
                for c in range(_NCH):
                    j0 = 1 if c == 0 else 0
                    j1 = 8 if c == _NCH - 1 else 9
                    xn_chunk = xn_t[s][c // 16][:, 128 * (c % 16) : 128 * (c % 16) + 128]
                    nc.tensor.matmul(
                        psC[:, 8 * c - 1 + j0 : 8 * c - 1 + j1],
                        xn_chunk,
                        S_t[s][:, 32 * j0 + c : 32 * (j1 - 1) + c + 1 : 32],
                        start=False,
                        stop=(c == _NCH - 1),
                        skip_group_check=True,
                    )

            def emit_out(s):
                # psum->bf16 eviction on ACT: keeps DVE free for folds and
                # makes the whole output chain (cast -> dispatch) a single
                # in-order ACT sequence with no cross-engine head-of-line
                outb = spool.tile([128, _NB], bf16, tag="outb", name=f"outb_{s}")
                nc.scalar.activation(outb[:], psC_t[s][:], AF.Copy)
                # output DMA on the ACT HWDGE ring -- idle once the last
                # fold/exp is done, well before the first psC is ready
                nc.scalar.dma_start(out[s, :, :], outb[:])

            # ---- schedule: all A/B chains first (xt's arrive first),
            # then the C chains chase the xn stream ----
            for s in range(_SL):
                emit_A(s)
                emit_fold(s)
                dmm = emit_den_mm(s)
                smm = emit_S_mm(s)
                emit_den_copy(s, dmm)
                emit_S_copy(s, smm)
            for s in range(_SL):
                emit_psC_memset(s)
                emit_C(s)
                emit_out(s)
            nc.scalar.dma_start(outd[0:1, :], den_all[:])

    nc.compile()
    return nc


def _get_program():
    if "nc" not in _prog_cache:
        _prog_cache["nc"] = _build_program()
    return _prog_cache["nc"]


def _host_inputs(x, W_gate):
    bf16 = ml_dtypes.bfloat16
    x = np.asarray(x, dtype=np.float32)
    W = np.asarray(W_gate, dtype=np.float32)
    # wt[d, k*32+g] = W_gate[g, k*128+d]
    wt_host = np.ascontiguousarray(
        W.reshape(_K, _K, _D).transpose(2, 1, 0).reshape(_D, _K * _K)
    ).astype(bf16)
    in_maps = []
    for core in range(_NC):
        xn = np.empty((_SL, 128, _NCH * _D), dtype=bf16)
        xt = np.zeros((_SL, 128, _NT), dtype=bf16)
        for si in range(_SL):
            p = core * _SL + si
            b, h = p // _H, p % _H
            xs = x[b, :, h, :]  # [4096, 128]
            xn[si] = (
                xs.reshape(_NCH, 128, _D).transpose(1, 0, 2).reshape(128, _NCH * _D)
            ).astype(bf16)
            xt[si, :, :_N] = xs.T.astype(bf16)
        in_maps.append({"xn": xn, "xt": xt, "wt": wt_host})
    return in_maps


def _assemble(results):
    out = np.empty((_B, _NB, _H, _D), dtype=np.float32)
    for core in range(_NC):
        o = np.asarray(results[core]["out"]).astype(np.float32)   # [SL, D, NB]
        dens = np.asarray(results[core]["outd"]).astype(np.float32).reshape(_SL, 256)
        for si in range(_SL):
            p = core * _SL + si
            out[p // _H, :, p % _H, :] = o[si].T / dens[si, :_NB, None]
    return out


def _install_trace_hooks():
    """Shim the axon NTFF profile hook (missing in this image) so
    run_bass_kernel_spmd(trace=True) can collect a HW profile, and neuter
    the artifact upload (zero-egress container)."""
    import contextlib
    import ctypes
    import types

    try:
        from antenv.axon_hooks import get_axon_ntff_profile_hook  # noqa: F401

        return
    except ImportError:
        pass

    lib = ctypes.CDLL("/opt/axon/libaxon_pjrt.so")
    if not hasattr(lib, "axon_start_nrt_profile"):
        return
    lib.axon_start_nrt_profile.argtypes = [
        ctypes.POINTER(ctypes.c_int64),
        ctypes.c_size_t,
    ]
    lib.axon_start_nrt_profile.restype = ctypes.c_int64
    lib.axon_stop_nrt_profile.argtypes = [ctypes.c_char_p]
    lib.axon_stop_nrt_profile.restype = ctypes.c_int64

    @contextlib.contextmanager
    def _hook(output_dir, device_ids):
        import jax

        jax.devices()
        if device_ids:
            ids = (ctypes.c_int64 * len(device_ids))(*device_ids)
            rc = lib.axon_start_nrt_profile(ids, len(device_ids))
        else:
            rc = lib.axon_start_nrt_profile(None, 0)
        if rc != 0:
            raise RuntimeError(f"axon_start_nrt_profile rc={rc}")
        try:
            yield
        finally:
            n = lib.axon_stop_nrt_profile(str(output_dir).encode())
            print(f"profile: {n} file(s) written to {output_dir}")

    mod = types.ModuleType("antenv.axon_hooks")
    mod.get_axon_ntff_profile_hook = lambda: _hook
    mod.set_axon_ntff_profile_hook = lambda h: None
    sys.modules["antenv.axon_hooks"] = mod

    from concourse import bass_utils as bu

    bu.upload_artifacts = lambda tmpdir: tmpdir


def run(x, W_gate, trace=False, **kw):
    from concourse.bass_utils import run_bass_kernel_spmd

    if trace:
        _install_trace_hooks()
    nc = _get_program()
    in_maps = _host_inputs(x, W_gate)
    res = run_bass_kernel_spmd(nc, in_maps, list(range(_NC)), trace=trace, **kw)
    return _assemble(res.results), res


def kernel(x, W_gate):
    out, _ = run(x, W_gate)
    return out


# revision 17
# speedup vs baseline: 1.1499x; 1.1499x over previous
"""CompressKV gating kernel for 8 Trainium2 NeuronCores.

Reference computation (per batch b, head h):
    x_s = x[b, :, h, :]                                  # [N=4096, D=128]
    windows n = 0..254, rows r = 16n + k, k = 0..31
    logits[n, g] = sum_{k,d} x_s[16n+k, d] * W[g, k, d]  # W = W_gate.reshape(32,32,128)
    gate = softmax_g(logits)
    out[n, d] = sum_k gate[n, k] * x_s[16n+k, d]

Sharding: B*H = 32 (b,h) slices, 4 per core, data/tensor parallel, no
cross-core communication.  Host pre-packs x per core in two bf16 layouts:
  xn: window-chunked native  [4, 128(p), 32(c)*128(d)]  (chunk c = rows 128c+p)
  xt: d-major (transposed)   [4, 128(d), 4224(n pad)]
plus the gate weight transposed to d-major wt[d, k*32+g].

HBM input traffic (8.5 MB/core, ~330 GB/s on the SP ring) is the hard
floor.  All input DMAs are issued up-front in an order matched to the
compute schedule (xt0 xt1 xn0 xt2 xn1 xt3 xn2 xn3) so the PE never waits
more than ~1.5 us and the HAM clock gate stays warm.

Softmax normalization is NOT done on device: the kernel ships the
unnormalized pooled output outT[d, n] = sum_k e[k,n] x[16n+k, d] (bf16)
plus the denominators den[n] = sum_k e[k,n] (f32), and the host divides.
This removes the recip/broadcast/mul chain from the critical path.

On-device pipeline per slice:
  A) logits via 32 accumulating matmuls (contract d chunks per k), 4 k's
     packed concurrently into the PE array via col-tiling -> psA[(kg,g), n]
  B) fold 4 col-groups (ACT copy + 3 DVE adds), exp (ACT) -> e[32, n] bf16,
     den via ones-matmul -> ACT copy to a packed SBUF strip.
  C) banded-matrix pooling: S[r, window] built from e with 9 band matmuls;
     32 matmuls with x chunks stationary -> psC[d, n]; DVE copy -> bf16;
     per-slice DMA on the DVE ring right after the copy.
PE instruction order interleaves slices (A_s | den/S_s | C_{s-1}) so
stage C of one slice fills the fold/exp latency of the next.
"""

import sys

import numpy as np

for _p in ("/opt/trn_rl_repo", "/opt/pypackages"):
    if _p not in sys.path:
        sys.path.append(_p)

import ml_dtypes

_B, _N, _H, _D = 2, 4096, 16, 128
_K = 32          # window (kernel) size
_ST = 16         # stride
_NB = 255        # num windows
_NC = 8          # cores
_SL = 4          # (b,h) slices per core
_NT = 4224       # padded n extent for xt (>= 16*255+31+1)
_NCH = 32        # 128-row chunks per slice

_prog_cache = {}


def _build_program():
    import concourse.mybir as mybir
    from concourse import bacc, tile

    f32 = mybir.dt.float32
    bf16 = mybir.dt.bfloat16
    AF = mybir.ActivationFunctionType

    nc = bacc.Bacc()
    xn = nc.dram_tensor("xn", [_SL, 128, _NCH * _D], bf16, kind="ExternalInput")
    xt = nc.dram_tensor("xt", [_SL, 128, _NT], bf16, kind="ExternalInput")
    wt = nc.dram_tensor("wt", [128, _K * _K], bf16, kind="ExternalInput")
    out = nc.dram_tensor("out", [_SL, _D, _NB], bf16, kind="ExternalOutput")
    outd = nc.dram_tensor("outd", [1, _SL * 256], f32, kind="ExternalOutput")

    with tile.TileContext(nc) as tc:
        with (
            tc.tile_pool(name="const", bufs=1) as cpool,
            tc.tile_pool(name="data", bufs=1) as dpool,
            tc.tile_pool(name="small", bufs=2) as spool,
            tc.tile_pool(name="psA", bufs=2, space="PSUM") as psa_pool,
            tc.tile_pool(name="psC", bufs=2, space="PSUM") as psc_pool,
            tc.tile_pool(name="psS", bufs=2, space="PSUM") as pss_pool,
            tc.tile_pool(name="psD", bufs=1, space="PSUM") as psd_pool,
            tc.tile_pool(name="psW", bufs=1, space="PSUM") as psw_pool,
        ):
            # ---- all input DMAs first, ordered to match the compute
            # schedule, so the SP ring streams back-to-back
            wt_sb = cpool.tile([128, _K * _K], bf16)
            nc.sync.dma_start(wt_sb[:], wt[:, :])
            # HAM warm-up: dependency-free matmuls on a memset constant tile
            # (ready ~4 us before the first DMA lands) keep the PE busy
            # through the preamble so the clock gate reaches 8/8 before the
            # first real matmul.
            wrm = cpool.tile([128, 128], bf16)
            nc.vector.memset(wrm[:], 0.5)
            psW = psw_pool.tile([32, 128], f32)
            for _w in range(45):
                nc.tensor.matmul(
                    psW[:, :], wrm[:, 0:32], wrm[:, :],
                    start=True, stop=True, skip_group_check=True,
                )
            xt_t = {
                s: [
                    dpool.tile([128, 2112], bf16, tag=f"xt{s}h{h}", name=f"xt{s}h{h}")
                    for h in range(2)
                ]
                for s in range(_SL)
            }
            xn_t = {
                s: [
                    dpool.tile([128, 16 * _D], bf16, tag=f"xn{s}h{h}", name=f"xn{s}h{h}")
                    for h in range(2)
                ]
                for s in range(_SL)
            }

            def load_xt(s):
                # xt half h covers window cols [2048h, 2048h+2112)
                nc.sync.dma_start(xt_t[s][0][:], xt[s, :, 0:2112])
                nc.sync.dma_start(xt_t[s][1][:], xt[s, :, 2048:4160])

            def load_xn(s):
                # halves keep the DMA partition lines at 4 KB (full rate);
                # stage C's chunk matmuls gate per-half via region deps
                nc.sync.dma_start(xn_t[s][0][:], xn[s, :, 0 : 16 * _D])
                nc.sync.dma_start(xn_t[s][1][:], xn[s, :, 16 * _D : 32 * _D])

            # all xt first: every logits/softmax chain completes with slack
            # while the xn stream is still arriving; the kernel tail is just
            # last-xn-quarter -> 8 pooling matmuls -> copy -> DMA
            for s in range(_SL):
                load_xt(s)
            for s in range(_SL):
                load_xn(s)

            # ---- constants
            ones32 = cpool.tile([32, 1], bf16)
            nc.vector.memset(ones32[:], 1.0)
            # band master: mband[k, c] = 1 iff c == k + 128.  Slicing cols
            # [144-16j : 272-16j] gives the [32k, 128r] band matrix with
            # 1 at r == k + 16j - 16 (rows outside [0,128) auto-dropped).
            mband = cpool.tile([32, 272], bf16)
            nc.gpsimd.memset(mband[:], 0.0)
            nc.gpsimd.affine_select(
                out=mband[:],
                in_=mband[:],
                compare_op=mybir.AluOpType.not_equal,
                fill=1.0,
                base=128,
                # fill where 128 + x - y == 0, i.e. y == x + 128
                pattern=[[-1, 272]],
                channel_multiplier=1,
            )
            den_all = cpool.tile([1, _SL * 256], f32)

            # ---- per-slice emitters (called in a hand-interleaved order) --
            psA_t, e_t, S_t, psC_t, outb_t = {}, {}, {}, {}, {}

            def emit_A(s):
                # logits via 32 accumulating matmuls per half (contract d
                # chunks per k), 4 k's packed concurrently into the PE
                # array via col-tiling (4-way streams measured ~2.5x the
                # serial rate, beating formulations with less traffic).
                psA = psa_pool.tile([128, 256], f32, tag="psA", name=f"psA_{s}")
                for h in range(2):
                    for t in range(8):
                        for j in range(4):
                            k = 4 * t + j
                            nc.tensor.matmul(
                                psA[32 * j : 32 * j + 32, 128 * h : 128 * h + 128],
                                wt_sb[:, 32 * k : 32 * k + 32],
                                xt_t[s][h][:, k : k + 16 * 128 : 16],
                                start=(t == 0),
                                stop=(t == 7),
                                tile_position=(0, 32 * j),
                                skip_group_check=True,
                            )
                psA_t[s] = psA

            def emit_fold(s):
                # fold the 4 col-groups: logits[g, n] = sum_j psA[32j+g, n]
                # (ACT evicts the first group; DVE may read at most one
                # PSUM operand per op.)  Then exp on ACT.
                # fold the 4 col-groups: logits[g, n] = sum_j psA[32j+g, n]
                # (ACT evicts the first group; DVE may read at most one PSUM
                # operand per op), then exp on ACT
                psA = psA_t[s]
                t0 = spool.tile([32, 256], f32, tag="t0", name=f"t0_{s}")
                t1 = spool.tile([32, 256], f32, tag="t1", name=f"t1_{s}")
                t2 = spool.tile([32, 256], f32, tag="t2", name=f"t2_{s}")
                logits = spool.tile([32, 256], f32, tag="lg", name=f"lg_{s}")
                nc.scalar.activation(t0[:], psA[0:32, :], AF.Copy)
                nc.vector.tensor_add(t1[:], t0[:], psA[32:64, :])
                nc.vector.tensor_add(t2[:], t1[:], psA[64:96, :])
                nc.vector.tensor_add(logits[:], t2[:], psA[96:128, :])
                e_kn = spool.tile([32, 256], bf16, tag="e", name=f"e_{s}")
                nc.scalar.activation(e_kn[:], logits[:], AF.Exp)
                e_t[s] = e_kn

            def emit_den_mm(s):
                psD = psd_pool.tile([32, 256], f32, tag="psD", name=f"psD_{s}")
                nc.tensor.matmul(
                    psD[0:1, :], ones32[:, 0:1], e_t[s][:, :],
                    start=True, stop=True,
                )
                return psD

            def emit_S_mm(s):
                # S matrix (class-major cols 32j + c): window n = 8c-1+j,
                # S[r=16j-16+k, 32j+c] = e[k, n].  Built on PE via band-
                # matrix lhsT (partition placement encoded in the matrix),
                # since engine partition bases must be 32-aligned.
                psS = pss_pool.tile([128, 9 * 32], f32, tag="psS", name=f"psS_{s}")
                for j in range(9):
                    c0 = 1 if j == 0 else 0
                    c1 = 31 if j == 8 else 32
                    nc.tensor.matmul(
                        psS[:, 32 * j + c0 : 32 * j + c1],
                        mband[:, 144 - 16 * j : 272 - 16 * j],
                        e_t[s][:, 8 * c0 + j - 1 : 8 * (c1 - 1) + j : 8],
                        start=True,
                        stop=True,
                        skip_group_check=True,
                    )
                return psS

            def emit_den_copy(s, psD):
                nc.scalar.activation(
                    den_all[0:1, 256 * s : 256 * s + 256], psD[0:1, :], AF.Copy
                )

            def emit_S_copy(s, psS):
                S_sb = spool.tile([128, 9 * 32], bf16, tag="S", name=f"S_{s}")
                # cols 0 and 287 are never written (invalid windows) nor read
                nc.vector.tensor_copy(S_sb[:, 1:287], psS[:, 1:287])
                S_t[s] = S_sb

            def emit_psC_memset(s):
                psC = psc_pool.tile([128, _NB], f32, tag="psC", name=f"psC_{s}")
                nc.vector.memset(psC[:], 0.0)
                psC_t[s] = psC

            def emit_C(s):
                psC = psC_t[s]
                for c in range(_NCH):
                    j0 = 1 if c == 0 else 0
                    j1 = 8 if c == _NCH - 1 else 9
                    xn_chunk = xn_t[s][c // 16][:, 128 * (c % 16) : 128 * (c % 16) + 128]
                    nc.tensor.matmul(
                        psC[:, 8 * c - 1 + j0 : 8 * c - 1 + j1],
                        xn_chunk,
                        S_t[s][:, 32 * j0 + c : 32 * (j1 - 1) + c + 1 : 32],
                        start=False,
                        stop=(c == _NCH - 1),
                        skip_group_check=True,
                    )

            def emit_out(s):
                # psum->bf16 eviction on ACT: keeps DVE free for folds and
                # makes the whole output chain (cast -> dispatch) a single
                # in-order ACT sequence with no cross-engine head-of-line
                outb = spool.tile([128, _NB], bf16, tag="outb", name=f"outb_{s}")
                nc.scalar.activation(outb[:], psC_t[s][:], AF.Copy)
                # output DMA on the ACT HWDGE ring -- idle once the last
                # fold/exp is done, well before the first psC is ready
                nc.scalar.dma_start(out[s, :, :], outb[:])

            # ---- schedule: all A/B chains first (xt's arrive first),
            # then the C chains chase the xn stream ----
            for s in range(_SL):
                emit_A(s)
                emit_fold(s)
                dmm = emit_den_mm(s)
                smm = emit_S_mm(s)
                emit_den_copy(s, dmm)
                emit_S_copy(s, smm)
            for s in range(_SL):
                emit_psC_memset(s)
                emit_C(s)
                emit_out(s)
            nc.scalar.dma_start(outd[0:1, :], den_all[:])

    nc.compile()
    return nc


def _get_program():
    if "nc" not in _prog_cache:
        _prog_cache["nc"] = _build_program()
    return _prog_cache["nc"]


def _host_inputs(x, W_gate):
    bf16 = ml_dtypes.bfloat16
    x = np.asarray(x, dtype=np.float32)
    W = np.asarray(W_gate, dtype=np.float32)
    # wt[d, k*32+g] = W_gate[g, k*128+d]
    wt_host = np.ascontiguousarray(
        W.reshape(_K, _K, _D).transpose(2, 1, 0).reshape(_D, _K * _K)
    ).astype(bf16)
    in_maps = []
    for core in range(_NC):
        xn = np.empty((_SL, 128, _NCH * _D), dtype=bf16)
        xt = np.zeros((_SL, 128, _NT), dtype=bf16)
        for si in range(_SL):
            p = core * _SL + si
            b, h = p // _H, p % _H
            xs = x[b, :, h, :]  # [4096, 128]
            xn[si] = (
                xs.reshape(_NCH, 128, _D).transpose(1, 0, 2).reshape(128, _NCH * _D)
            ).astype(bf16)
            xt[si, :, :_N] = xs.T.astype(bf16)
        in_maps.append({"xn": xn, "xt": xt, "wt": wt_host})
    return in_maps


def _assemble(results):
    out = np.empty((_B, _NB, _H, _D), dtype=np.float32)
    for core in range(_NC):
        o = np.asarray(results[core]["out"]).astype(np.float32)   # [SL, D, NB]
        dens = np.asarray(results[core]["outd"]).astype(np.float32).reshape(_SL, 256)
        for si in range(_SL):
            p = core * _SL + si
            out[p // _H, :, p % _H, :] = o[si].T / dens[si, :_NB, None]
    return out


def _install_trace_hooks():
    """Shim the axon NTFF profile hook (missing in this image) so
    run_bass_kernel_spmd(trace=True) can collect a HW profile, and neuter
    the artifact upload (zero-egress container)."""
    import contextlib
    import ctypes
    import types

    try:
        from antenv.axon_hooks import get_axon_ntff_profile_hook  # noqa: F401

        return
    except ImportError:
        pass

    lib = ctypes.CDLL("/opt/axon/libaxon_pjrt.so")
    if not hasattr(lib, "axon_start_nrt_profile"):
        return
    lib.axon_start_nrt_profile.argtypes = [
        ctypes.POINTER(ctypes.c_int64),
        ctypes.c_size_t,
    ]
    lib.axon_start_nrt_profile.restype = ctypes.c_int64
    lib.axon_stop_nrt_profile.argtypes = [ctypes.c_char_p]
    lib.axon_stop_nrt_profile.restype = ctypes.c_int64

    @contextlib.contextmanager
    def _hook(output_dir, device_ids):
        import jax

        jax.devices()
        if device_ids:
            ids = (ctypes.c_int64 * len(device_ids))(*device_ids)
            rc = lib.axon_start_nrt_profile(ids, len(device_ids))
        else:
            rc = lib.axon_start_nrt_profile(None, 0)
        if rc != 0:
            raise RuntimeError(f"axon_start_nrt_profile rc={rc}")
        try:
            yield
        finally:
            n = lib.axon_stop_nrt_profile(str(output_dir).encode())
            print(f"profile: {n} file(s) written to {output_dir}")

    mod = types.ModuleType("antenv.axon_hooks")
    mod.get_axon_ntff_profile_hook = lambda: _hook
    mod.set_axon_ntff_profile_hook = lambda h: None
    sys.modules["antenv.axon_hooks"] = mod

    from concourse import bass_utils as bu

    bu.upload_artifacts = lambda tmpdir: tmpdir


def run(x, W_gate, trace=False, **kw):
    from concourse.bass_utils import run_bass_kernel_spmd

    if trace:
        _install_trace_hooks()
    nc = _get_program()
    in_maps = _host_inputs(x, W_gate)
    res = run_bass_kernel_spmd(nc, in_maps, list(range(_NC)), trace=trace, **kw)
    return _assemble(res.results), res


def kernel(x, W_gate):
    out, _ = run(x, W_gate)
    return out
